# revision 17
# baseline (speedup 1.0000x reference)
"""Multi-head attention (B=4, S=2048, H=1024, 16 heads) on 8 TRN2 NeuronCores.

Sharding: core c handles (batch b = c//2, head-group g = c%2 of 8 heads).
Per-core device program (all activations kept feature-major, i.e. transposed):
  X^T  [1024,2048]  via DMA-xbar-transpose of the bf16-cast query shard
  Q^T,K^T = Wq/k^T X^T            (PE, bf16, fp32 PSUM)
  V       = X W_v  (natural [s,d] via lhsT=X^T tiles)
  S^T  = K^T.T-style: per head, per k-tile: lhsT=K^T slice, rhs=Q^T slice
  E    = exp(S^T/8)  on ScalarE, bf16 out
  O^T,sums = V'.T @ E  where V' = [V | 1] (ones column yields softmax sums)
  O^T /= sums (reciprocal + gpsimd partition-broadcast + DVE mul)
  Y^T  = W_o^T O^T   -> DMA out per [128,512] tile
Host: gathers per-core Y^T tiles, sums the two head-group partials per batch,
adds b_o. b_qkv / attention bias are zero in this problem; nonzero values are
still handled (extra rank-1 bias matmuls / DVE bias adds) via build flags.
"""

import numpy as np
import ml_dtypes

import concourse.bass as bass
import concourse.tile as tile
from concourse import bacc
import concourse.mybir as mybir
from concourse.bass_utils import run_bass_kernel_spmd

F32 = mybir.dt.float32
BF16 = mybir.dt.bfloat16
AF = mybir.ActivationFunctionType

HIDDEN = 1024
HEADS = 16
HD = 64
B = 4
S_FULL = 2048
NCORES = 8
HPG = HEADS // 2          # heads per group/core = 8
GF = HPG * HD             # group feature width = 512
NPAIR = HPG // 2          # head pairs per core = 4


def build_program(S=S_FULL, has_bqkv=False, has_bias=False):
    KT = HIDDEN // 128            # hidden k-tiles = 8
    CH = min(512, S)              # free-dim chunk
    NQC = S // CH                 # q chunks
    SKT = S // 128                # seq k-tiles (attention contraction)
    NM = HIDDEN // 128            # output-projection m-tiles = 8

    nc = bacc.Bacc(
        "TRN2",
        target_bir_lowering=False,
        debug=False,
        enable_asserts=False,
        num_devices=NCORES,
    )

    x_dram = nc.dram_tensor("x", [HIDDEN, S], BF16, kind="ExternalInput")  # X^T, host-transposed
    wqkv_dram = nc.dram_tensor("wqkv", [HIDDEN, 3 * GF], BF16, kind="ExternalInput")
    wo_dram = nc.dram_tensor("wo", [GF, HIDDEN], BF16, kind="ExternalInput")
    if has_bqkv:
        bqkv_dram = nc.dram_tensor("bqkv", [1, 3 * GF], BF16, kind="ExternalInput")
    if has_bias:
        # host passes bias[0,0].T * 8 so exp(0.125*(S + bias8)) = exp(S/8 + bias)
        bias8_dram = nc.dram_tensor("bias8t", [S, S], F32, kind="ExternalInput")
    y_dram = nc.dram_tensor("y", [NM, NQC, 128, CH], F32, kind="ExternalOutput")

    with tile.TileContext(nc) as tc:
        with (
            tc.tile_pool(name="res", bufs=1) as res,
            tc.tile_pool(name="wrk", bufs=2) as wrk,
            tc.tile_pool(name="ep", bufs=4) as ep,
            tc.tile_pool(name="ps", bufs=2, space="PSUM") as ps,
        ):
            xt = res.tile([128, KT * S], BF16, tag="xt")
            wqkv = res.tile([128, KT * 3 * GF], BF16, tag="wqkv")
            wo = res.tile([128, (GF // 128) * HIDDEN], BF16, tag="wo")
            # V' tiles: per s-tile block of 8 heads x 65 cols (65th col = 1.0)
            vp = res.tile([128, SKT * HPG * 65], BF16, tag="vp")
            ot = res.tile([128, NPAIR * S], BF16, tag="ot")

            nc.vector.memset(vp[:, :], 1.0)

            for kt in range(KT):
                nc.sync.dma_start(
                    xt[:, kt * S:(kt + 1) * S], x_dram[kt * 128:(kt + 1) * 128, :]
                )
                nc.scalar.dma_start(
                    wqkv[:, kt * 3 * GF:(kt + 1) * 3 * GF],
                    wqkv_dram[kt * 128:(kt + 1) * 128, :],
                )
            for ft in range(GF // 128):
                nc.scalar.dma_start(
                    wo[:, ft * HIDDEN:(ft + 1) * HIDDEN],
                    wo_dram[ft * 128:(ft + 1) * 128, :],
                )
            if has_bqkv:
                bq = res.tile([1, 3 * GF], BF16, tag="bq")
                nc.sync.dma_start(bq[:, :], bqkv_dram[:, :])
                ones = res.tile([1, CH], BF16, tag="ones")
                nc.vector.memset(ones[:, :], 1.0)

            def acc_matmul(out_ps, lhsT_of, rhs_of, bias_lhsT, bias_rhs):
                """Accumulate KT matmuls (+ optional rank-1 bias term) into PSUM."""
                if bias_lhsT is not None:
                    nc.tensor.matmul(out_ps, bias_lhsT, bias_rhs, start=True, stop=False)
                for kt in range(KT):
                    nc.tensor.matmul(
                        out_ps,
                        lhsT_of(kt),
                        rhs_of(kt),
                        start=(kt == 0 and bias_lhsT is None),
                        stop=(kt == KT - 1),
                    )

            # ---- V phase: V' for all heads, natural [s, d] layout ----
            for st in range(SKT):
                vps = ps.tile([128, GF], F32, tag="sp", bufs=2)
                acc_matmul(
                    vps[:, :],
                    lambda kt, st=st: xt[:, kt * S + st * 128: kt * S + (st + 1) * 128],
                    lambda kt: wqkv[:, kt * 3 * GF + 2 * GF: kt * 3 * GF + 3 * GF],
                    ones[0:1, 0:128] if has_bqkv else None,
                    bq[0:1, 2 * GF:3 * GF] if has_bqkv else None,
                )
                dst = vp[:, st * HPG * 65:(st + 1) * HPG * 65]
                dst = dst.rearrange("p (h c) -> p h c", c=65)[:, :, 0:64]
                src = vps.rearrange("p (h c) -> p h c", c=64)
                nc.vector.tensor_copy(dst, src)

            # ---- per head-pair: Q^T, K^T then attention ----
            for p in range(NPAIR):
                qt = wrk.tile([128, S], BF16, tag="qt")
                ktt = wrk.tile([128, S], BF16, tag="ktt")
                for dst_sb, colbase in ((qt, p * 128), (ktt, GF + p * 128)):
                    for qc in range(NQC):
                        qkps = ps.tile([128, CH], F32, tag="sp", bufs=2)
                        acc_matmul(
                            qkps[:, :],
                            lambda kt, cb=colbase: wqkv[:, kt * 3 * GF + cb: kt * 3 * GF + cb + 128],
                            lambda kt, qc=qc: xt[:, kt * S + qc * CH: kt * S + (qc + 1) * CH],
                            bq[0:1, colbase:colbase + 128] if has_bqkv else None,
                            ones[0:1, 0:CH] if has_bqkv else None,
                        )
                        nc.vector.tensor_copy(dst_sb[:, qc * CH:(qc + 1) * CH], qkps[:, :])

                for qc in range(NQC):
                    avs = []
                    for j in (0, 1):
                        avp = ps.tile([65, CH], F32, tag="av", bufs=4)
                        avs.append(avp)

                    def emit_av(e_tile, st, p=p, avs=avs):
                        for j in (0, 1):
                            h = p * 2 + j
                            nc.tensor.matmul(
                                avs[j][:, :],
                                vp[:, st * HPG * 65 + h * 65: st * HPG * 65 + (h + 1) * 65],
                                e_tile[:, j * CH:(j + 1) * CH],
                                start=(st == 0),
                                stop=(st == SKT - 1),
                            )

                    # fused path only when each head's chunk is a full PSUM
                    # bank: matmul outputs must be bank-aligned
                    fused = (CH == 512)
                    pending = None  # (e_tile, st) — AV emitted one k-tile late
                    for st in range(SKT):
                        e = ep.tile([128, 2 * CH], BF16, tag="e", bufs=6)
                        if fused:
                            sps = [None]
                            sp = ps.tile([128, 2 * CH], F32, tag="sp", bufs=2)
                        for j in (0, 1):
                            hs = slice(j * 64, (j + 1) * 64)
                            if fused:
                                spv = sp[:, j * CH:(j + 1) * CH]
                            else:
                                spj = ps.tile([128, CH], F32, tag="sp", bufs=2)
                                spv = spj[:, :]
                            nc.tensor.matmul(
                                spv,
                                ktt[hs, st * 128:(st + 1) * 128],
                                qt[hs, qc * CH:(qc + 1) * CH],
                                start=True,
                                stop=True,
                                tile_position=(j * 64, 0),
                            )
                            if has_bias:
                                b8 = ep.tile([128, CH], F32, tag="b8", bufs=2)
                                nc.sync.dma_start(
                                    b8[:, :],
                                    bias8_dram[st * 128:(st + 1) * 128, qc * CH:(qc + 1) * CH],
                                )
                                nc.vector.tensor_add(spv, spv, b8[:, :])
                            if not fused:
                                nc.scalar.activation(
                                    e[:, j * CH:(j + 1) * CH], spv, AF.Exp, scale=0.125
                                )
                        if fused:
                            nc.scalar.activation(e[:, :], sp[:, :], AF.Exp, scale=0.125)
                        if pending is not None:
                            emit_av(*pending)
                        pending = (e, st)
                    emit_av(*pending)
                    # fast PSUM->SBUF copies release both accumulator slots
                    # first; the normalize chain (recip/bcast/mul) then runs
                    # off the PE critical path. GpSimd runs ONLY
                    # partition_broadcast (mixing gpsimd op types causes a
                    # ~6us library reload per op).
                    raws = []
                    for j in (0, 1):
                        raw = wrk.tile([65, CH], F32, tag="raw", bufs=4)
                        nc.vector.tensor_copy(raw[:, :], avs[j][:, :])
                        raws.append(raw)
                    for j in (0, 1):
                        rec = wrk.tile([1, CH], F32, tag="rec")
                        nc.vector.reciprocal(rec[:, :], raws[j][64:65, :])
                        bc = wrk.tile([64, CH], F32, tag="bc")
                        nc.gpsimd.partition_broadcast(bc[:, :], rec[:, :])
                        nc.vector.tensor_mul(
                            ot[j * 64:(j + 1) * 64, p * S + qc * CH: p * S + (qc + 1) * CH],
                            raws[j][0:64, :],
                            bc[:, :],
                        )

            # ---- output projection: Y^T[e, q] ----
            for m in range(NM):
                for qc in range(NQC):
                    yps = ps.tile([128, CH], F32, tag="sp", bufs=2)
                    for ft in range(GF // 128):
                        nc.tensor.matmul(
                            yps[:, :],
                            wo[:, ft * HIDDEN + m * 128: ft * HIDDEN + (m + 1) * 128],
                            ot[:, ft * S + qc * CH: ft * S + (qc + 1) * CH],
                            start=(ft == 0),
                            stop=(ft == GF // 128 - 1),
                        )
                    ysb = wrk.tile([128, CH], F32, tag="ysb")
                    nc.vector.tensor_copy(ysb[:, :], yps[:, :])
                    nc.sync.dma_start(y_dram[m, qc], ysb[:, :])

    nc.compile()
    return nc


_BUILD_CACHE = {}


def _get_program(S, has_bqkv, has_bias):
    key = (S, has_bqkv, has_bias)
    if key not in _BUILD_CACHE:
        _BUILD_CACHE[key] = build_program(S, has_bqkv, has_bias)
    return _BUILD_CACHE[key]


def make_in_maps(query, bias, w_qkv, b_qkv, w_o, has_bqkv, has_bias):
    bf = ml_dtypes.bfloat16
    in_maps = []
    for c in range(NCORES):
        b, g = divmod(c, 2)
        cols = slice(g * GF, (g + 1) * GF)
        w_g = np.concatenate(
            [w_qkv[:, cols], w_qkv[:, HIDDEN:][:, cols], w_qkv[:, 2 * HIDDEN:][:, cols]],
            axis=1,
        )
        m = {
            "x": np.ascontiguousarray(query[b].T).astype(bf),
            "wqkv": np.ascontiguousarray(w_g).astype(bf),
            "wo": np.ascontiguousarray(w_o[cols]).astype(bf),
        }
        if has_bqkv:
            b_g = np.concatenate(
                [b_qkv[cols], b_qkv[HIDDEN:][cols], b_qkv[2 * HIDDEN:][cols]]
            )
            m["bqkv"] = b_g.reshape(1, 3 * GF).astype(bf)
        if has_bias:
            m["bias8t"] = np.ascontiguousarray(bias[0, 0].T * 8.0).astype(np.float32)
        in_maps.append(m)
    return in_maps


def assemble_output(results, b_o, S=S_FULL):
    NQC = S // min(512, S)
    out = np.zeros((B, S, HIDDEN), np.float32)
    for c in range(NCORES):
        b, _g = divmod(c, 2)
        y = results[c]["y"]  # [NM, NQC, 128, CH]
        yt = y.transpose(0, 2, 1, 3).reshape(HIDDEN, S)
        out[b] += yt.T
    out += np.asarray(b_o, np.float32)[None, None, :]
    return out


def kernel(query, bias, w_qkv, b_qkv, w_o, b_o, _trace=False):
    query = np.asarray(query, np.float32)
    bias = np.asarray(bias, np.float32)
    w_qkv = np.asarray(w_qkv, np.float32)
    b_qkv = np.asarray(b_qkv, np.float32)
    w_o = np.asarray(w_o, np.float32)
    b_o = np.asarray(b_o, np.float32)

    has_bqkv = bool(np.any(b_qkv))
    has_bias = bool(np.any(bias))
    nc = _get_program(S_FULL, has_bqkv, has_bias)
    in_maps = make_in_maps(query, bias, w_qkv, b_qkv, w_o, has_bqkv, has_bias)
    res = run_bass_kernel_spmd(
        nc, in_maps, core_ids=list(range(NCORES)), trace=_trace
    )
    out = assemble_output(res.results, b_o)
    if _trace:
        return out, res
    return out
